# revision 10
# baseline (speedup 1.0000x reference)
"""BinaryLinear (binarized nn.Linear) on 8 Trainium2 NeuronCores.

Reference op:
    alpha = mean(|W|, axis=1)                # per-output-row scale
    BW    = sign(W) * alpha                  # sign(0) := +1
    Y     = einsum('bsi,oi->bso', X, BW) + bias

Distribution: data-parallel over the batch dim (8 batches -> 1 per core).
Each core receives its batch slice of X pre-transposed (xT = [in, tok]),
the full weight in both layouts (wT = [in, out] for the matmul stationary
operand, w = [out, in] for the per-row alpha reduction), and bias. Each
core computes the full [tok, out] output for its batch element (stored
transposed as [out, tok]); the host transposes back and stacks.

On-device per core:
  - sign half-trick: s = (w >= 0) - 0.5 in {+0.5, -0.5} (one DVE op, exact
    in every dtype); the missing x2 is folded into alpha2 = 2*mean|W|.
  - alpha: DVE abs-accumulate reduce over natural-layout weight rows.
  - matmul: fp32r (full-rate fp32 PE mode), K=2048 accumulated in PSUM.
    Out-chunks are processed in PAIRS with the k-chunk loop OUTERMOST so
    each arriving x-chunk unblocks 8 matmuls (all 8 PSUM banks) -- this
    hides the initial 16 MiB x load behind PE work.
  - epilogue: one ScalarE activation per psum tile:
    Identity(psum*alpha2 + bias), then DMA out.
"""

import os

import numpy as np

B, T, K, O = 8, 2048, 2048, 2048  # batch, tokens, in_features, out_features
P = 128          # SBUF partitions
KC = K // P      # 16 k-chunks
OC = O // P      # 16 out-chunks
TN = 512         # moving free-dim per matmul (fp32 max)
TT = T // TN     # 4 token tiles

N_CORES = 8

# Stashed by kernel() for test harnesses: BassKernelResults of the last run.
last_results = None

_cached_nc = None


def _build_program():
    global _cached_nc
    if _cached_nc is not None:
        return _cached_nc

    import concourse.tile as tile
    from concourse import bacc, mybir

    F32 = mybir.dt.float32
    F32R = mybir.dt.float32r
    IDENT = mybir.ActivationFunctionType.Identity
    ALU = mybir.AluOpType
    AX = mybir.AxisListType

    nc = bacc.Bacc("TRN2", target_bir_lowering=False, debug=False,
                   num_devices=N_CORES)

    # x arrives pre-transposed and is consumed as the (reduced-precision)
    # f32r moving operand directly -- no on-chip cast pass.
    xT = nc.dram_tensor("xT", [K, T], F32R, kind="ExternalInput").ap()
    wT = nc.dram_tensor("wT", [K, O], F32, kind="ExternalInput").ap()
    w = nc.dram_tensor("w", [O, K], F32, kind="ExternalInput").ap()
    b = nc.dram_tensor("b", [O], F32, kind="ExternalInput").ap()
    yT = nc.dram_tensor("yT", [O, T], F32, kind="ExternalOutput").ap()

    xT_r = xT.rearrange("(c p) t -> p c t", p=P)
    wT_r = wT.rearrange("(c p) o -> p c o", p=P)

    with tile.TileContext(nc) as tc:
        with (
            tc.tile_pool(name="xpool", bufs=1) as xpool,
            tc.tile_pool(name="wpool", bufs=2) as wpool,
            tc.tile_pool(name="spool", bufs=3) as spool,
            tc.tile_pool(name="npool", bufs=2) as npool,
            tc.tile_pool(name="apool", bufs=4) as apool,
            tc.tile_pool(name="opool", bufs=3) as opool,
            tc.tile_pool(name="const", bufs=1) as const,
            tc.tile_pool(name="psum", bufs=8, space="PSUM") as psum,
        ):
            def sign_prep(o):
                """Load + binarize the stationary operand for out-chunk o."""
                wraw = wpool.tile([P, KC, P], F32, tag="wraw",
                                  name=f"wraw{o}")
                nc.sync.dma_start(out=wraw, in_=wT_r[:, :, o * P:(o + 1) * P])
                sw = spool.tile([P, KC, P], F32R, tag="sw", name=f"sw{o}")
                nc.vector.tensor_scalar(sw, wraw, 0.0, 0.5,
                                        op0=ALU.is_ge, op1=ALU.subtract)
                return sw

            def alpha_prep(o):
                """alpha2 = 2*mean|W_row| for out-chunk o, [128,1]."""
                wn = npool.tile([P, K], F32, tag="wn", name=f"wn{o}")
                nc.sync.dma_start(out=wn, in_=w[o * P:(o + 1) * P, :])
                asum = apool.tile([P, 1], F32, tag="asum", name=f"as{o}")
                nc.vector.tensor_reduce(asum, wn, axis=AX.X, op=ALU.add,
                                        apply_absolute_value=True)
                alpha2 = apool.tile([P, 1], F32, tag="alpha2", name=f"al{o}")
                nc.vector.tensor_scalar_mul(alpha2, asum, 2.0 / K)
                return alpha2

            def weight_prep(o):
                return sign_prep(o), alpha_prep(o)

            # only what the first matmuls need goes ahead of the x stream
            early_sw = {0: sign_prep(0), 1: sign_prep(1)}

            # resident x: 16 chunk tiles [128, 2048] f32r (i on partitions)
            x_tiles = []
            for c in range(KC):
                xt = xpool.tile([P, T], F32R, tag=f"x{c}")
                nc.sync.dma_start(out=xt, in_=xT_r[:, c, :])
                x_tiles.append(xt)

            # bias as [128, 16]: column c holds bias[c*128 : (c+1)*128];
            # epilogue-only data, loaded behind x
            bias_sb = const.tile([P, OC], F32)
            nc.sync.dma_start(out=bias_sb, in_=b.rearrange("(c p) -> p c", p=P))

            prepped = {0: (early_sw.pop(0), alpha_prep(0)),
                       1: (early_sw.pop(1), alpha_prep(1))}

            for pair in range(OC // 2):
                o0, o1 = 2 * pair, 2 * pair + 1
                pair_w = [prepped.pop(o0), prepped.pop(o1)]

                ps = [psum.tile([P, TN], F32, tag="ps", name=f"ps{pair}_{i}")
                      for i in range(8)]

                def epilogue(j, t):
                    o = (o0, o1)[j]
                    ot = opool.tile([P, TN], F32, tag="ot", name=f"ot{pair}_{j}_{t}")
                    nc.scalar.activation(ot, ps[j * TT + t], IDENT,
                                         bias=bias_sb[:, o:o + 1],
                                         scale=pair_w[j][1])
                    nc.sync.dma_start(
                        out=yT[o * P:(o + 1) * P, t * TN:(t + 1) * TN],
                        in_=ot)

                if pair < 2:
                    # x still streaming in: k-chunk outermost so every
                    # arriving x chunk unblocks 8 matmuls (all psum banks)
                    for c in range(KC):
                        for j in range(2):
                            sw = pair_w[j][0]
                            for t in range(TT):
                                nc.tensor.matmul(
                                    ps[j * TT + t],
                                    lhsT=sw[:, c, :],
                                    rhs=x_tiles[c][:, t * TN:(t + 1) * TN],
                                    start=(c == 0),
                                    stop=(c == KC - 1),
                                )
                    for j in range(2):
                        for t in range(TT):
                            epilogue(j, t)
                else:
                    # steady state: one psum group at a time so groups finish
                    # staggered -- banks free incrementally and epilogues
                    # overlap the next group's matmuls
                    for j in range(2):
                        sw = pair_w[j][0]
                        for t in range(TT):
                            for c in range(KC):
                                nc.tensor.matmul(
                                    ps[j * TT + t],
                                    lhsT=sw[:, c, :],
                                    rhs=x_tiles[c][:, t * TN:(t + 1) * TN],
                                    start=(c == 0),
                                    stop=(c == KC - 1),
                                )
                            epilogue(j, t)

                # prefetch next pair's weights (after this pair's matmul
                # emission so the DMAs sit behind x chunks in issue order)
                if pair + 1 < OC // 2:
                    prepped[2 * pair + 2] = weight_prep(2 * pair + 2)
                    prepped[2 * pair + 3] = weight_prep(2 * pair + 3)

    nc.compile()
    _cached_nc = nc
    return nc


def _make_in_maps(x, weight, bias):
    wT = np.ascontiguousarray(weight.T)
    w = np.ascontiguousarray(weight)
    b = np.ascontiguousarray(bias)
    in_maps = []
    for core in range(N_CORES):
        xb = np.ascontiguousarray(x[core].T)  # [in, tok]
        in_maps.append({"xT": xb, "wT": wT, "w": w, "b": b})
    return in_maps


def _setup_trace_hooks():
    """Provide the antenv.axon_hooks NTFF hook missing from this image and
    skip the artifact bucket upload so trace=True works locally."""
    import sys
    import types

    try:
        from antenv.axon_hooks import get_axon_ntff_profile_hook  # noqa: F401
    except ImportError:
        mod = types.ModuleType("antenv.axon_hooks")
        _h = [None]
        mod.set_axon_ntff_profile_hook = lambda h: _h.__setitem__(0, h)
        mod.get_axon_ntff_profile_hook = lambda: _h[0]
        sys.modules["antenv.axon_hooks"] = mod
        import antenv

        antenv.axon_hooks = mod
        from trn_agent_boot.trn_boot import _ntff_profile_via_ctypes

        mod.set_axon_ntff_profile_hook(
            _ntff_profile_via_ctypes("/opt/axon/libaxon_pjrt.so"))

    import concourse.bass_utils as bu

    bu.upload_artifacts = lambda tmpdir: f"local://{tmpdir}"


def kernel(x: np.ndarray, weight: np.ndarray, bias: np.ndarray) -> np.ndarray:
    global last_results
    from concourse.bass_utils import run_bass_kernel_spmd

    x = np.asarray(x, dtype=np.float32)
    weight = np.asarray(weight, dtype=np.float32)
    bias = np.asarray(bias, dtype=np.float32)

    nc = _build_program()
    in_maps = _make_in_maps(x, weight, bias)
    trace = bool(int(os.environ.get("KERNEL_TRACE", "0")))
    trace_cores = None
    if trace:
        _setup_trace_hooks()
        tc_env = os.environ.get("KERNEL_TRACE_CORES", "")
        if tc_env:
            trace_cores = [int(c) for c in tc_env.split(",")]
    res = run_bass_kernel_spmd(nc, in_maps, list(range(N_CORES)), trace=trace,
                               trace_cores=trace_cores)
    last_results = res

    out = np.empty((B, T, O), dtype=np.float32)
    for core in range(N_CORES):
        out[core] = res.results[core]["yT"].T
    return out


# revision 11
# speedup vs baseline: 1.0440x; 1.0440x over previous
"""BinaryLinear (binarized nn.Linear) on 8 Trainium2 NeuronCores.

Reference op:
    alpha = mean(|W|, axis=1)                # per-output-row scale
    BW    = sign(W) * alpha                  # sign(0) := +1
    Y     = einsum('bsi,oi->bso', X, BW) + bias

Distribution: data-parallel over the batch dim (8 batches -> 1 per core).
Each core receives its batch slice of X pre-transposed (xT = [in, tok]),
the full weight in both layouts (wT = [in, out] for the matmul stationary
operand, w = [out, in] for the per-row alpha reduction), and bias. Each
core computes the full [tok, out] output for its batch element (stored
transposed as [out, tok]); the host transposes back and stacks.

On-device per core:
  - sign half-trick: s = (w >= 0) - 0.5 in {+0.5, -0.5} (one DVE op, exact
    in every dtype); the missing x2 is folded into alpha2 = 2*mean|W|.
  - alpha: DVE abs-accumulate reduce over natural-layout weight rows.
  - matmul: fp32r (full-rate fp32 PE mode), K=2048 accumulated in PSUM.
    Out-chunks are processed in PAIRS with the k-chunk loop OUTERMOST so
    each arriving x-chunk unblocks 8 matmuls (all 8 PSUM banks) -- this
    hides the initial 16 MiB x load behind PE work.
  - epilogue: one ScalarE activation per psum tile:
    Identity(psum*alpha2 + bias), then DMA out.
"""

import os

import numpy as np

B, T, K, O = 8, 2048, 2048, 2048  # batch, tokens, in_features, out_features
P = 128          # SBUF partitions
KC = K // P      # 16 k-chunks
OC = O // P      # 16 out-chunks
TN = 512         # moving free-dim per matmul (fp32 max)
TT = T // TN     # 4 token tiles

N_CORES = 8

# Stashed by kernel() for test harnesses: BassKernelResults of the last run.
last_results = None

_cached_nc = None


def _build_program():
    global _cached_nc
    if _cached_nc is not None:
        return _cached_nc

    import concourse.tile as tile
    from concourse import bacc, mybir

    F32 = mybir.dt.float32
    F32R = mybir.dt.float32r
    IDENT = mybir.ActivationFunctionType.Identity
    ALU = mybir.AluOpType
    AX = mybir.AxisListType

    nc = bacc.Bacc("TRN2", target_bir_lowering=False, debug=False,
                   num_devices=N_CORES)

    # x arrives pre-transposed and is consumed as the (reduced-precision)
    # f32r moving operand directly -- no on-chip cast pass.
    xT = nc.dram_tensor("xT", [K, T], F32R, kind="ExternalInput").ap()
    wT = nc.dram_tensor("wT", [K, O], F32, kind="ExternalInput").ap()
    w = nc.dram_tensor("w", [O, K], F32, kind="ExternalInput").ap()
    b = nc.dram_tensor("b", [O], F32, kind="ExternalInput").ap()
    yT = nc.dram_tensor("yT", [O, T], F32, kind="ExternalOutput").ap()

    xT_r = xT.rearrange("(c p) t -> p c t", p=P)
    wT_r = wT.rearrange("(c p) o -> p c o", p=P)

    with tile.TileContext(nc) as tc:
        with (
            tc.tile_pool(name="xpool", bufs=1) as xpool,
            tc.tile_pool(name="wpool", bufs=2) as wpool,
            tc.tile_pool(name="spool", bufs=3) as spool,
            tc.tile_pool(name="npool", bufs=2) as npool,
            tc.tile_pool(name="apool", bufs=4) as apool,
            tc.tile_pool(name="opool", bufs=3) as opool,
            tc.tile_pool(name="const", bufs=1) as const,
            tc.tile_pool(name="psum", bufs=8, space="PSUM") as psum,
        ):
            def sign_prep(o):
                """Load + binarize the stationary operand for out-chunk o."""
                wraw = wpool.tile([P, KC, P], F32, tag="wraw",
                                  name=f"wraw{o}")
                nc.sync.dma_start(out=wraw, in_=wT_r[:, :, o * P:(o + 1) * P])
                sw = spool.tile([P, KC, P], F32R, tag="sw", name=f"sw{o}")
                nc.vector.tensor_scalar(sw, wraw, 0.0, 0.5,
                                        op0=ALU.is_ge, op1=ALU.subtract)
                return sw

            def alpha_prep(o):
                """alpha2 = 2*mean|W_row| for out-chunk o, [128,1]."""
                wn = npool.tile([P, K], F32, tag="wn", name=f"wn{o}")
                nc.sync.dma_start(out=wn, in_=w[o * P:(o + 1) * P, :])
                asum = apool.tile([P, 1], F32, tag="asum", name=f"as{o}")
                nc.vector.tensor_reduce(asum, wn, axis=AX.X, op=ALU.add,
                                        apply_absolute_value=True)
                alpha2 = apool.tile([P, 1], F32, tag="alpha2", name=f"al{o}")
                nc.vector.tensor_scalar_mul(alpha2, asum, 2.0 / K)
                return alpha2

            def weight_prep(o):
                return sign_prep(o), alpha_prep(o)

            # only what the first matmuls need goes ahead of the x stream
            early_sw = {0: sign_prep(0), 1: sign_prep(1)}

            # resident x: 16 chunk tiles [128, 2048] f32r (i on partitions).
            # Epilogue-only data (alpha rows, bias) is interleaved after the
            # first few chunks: early enough for pair-0 epilogues (~45us),
            # late enough not to delay the first matmuls.
            x_tiles = []
            early_alpha = {}
            bias_sb = None
            for c in range(KC):
                xt = xpool.tile([P, T], F32R, tag=f"x{c}")
                nc.sync.dma_start(out=xt, in_=xT_r[:, c, :])
                x_tiles.append(xt)
                if c == 3:
                    early_alpha = {0: alpha_prep(0), 1: alpha_prep(1)}
                    bias_sb = const.tile([P, OC], F32)
                    nc.sync.dma_start(out=bias_sb,
                                      in_=b.rearrange("(c p) -> p c", p=P))

            prepped = {0: (early_sw.pop(0), early_alpha.pop(0)),
                       1: (early_sw.pop(1), early_alpha.pop(1))}

            for pair in range(OC // 2):
                o0, o1 = 2 * pair, 2 * pair + 1
                pair_w = [prepped.pop(o0), prepped.pop(o1)]

                ps = [psum.tile([P, TN], F32, tag="ps", name=f"ps{pair}_{i}")
                      for i in range(8)]

                def epilogue(j, t):
                    o = (o0, o1)[j]
                    ot = opool.tile([P, TN], F32, tag="ot", name=f"ot{pair}_{j}_{t}")
                    nc.scalar.activation(ot, ps[j * TT + t], IDENT,
                                         bias=bias_sb[:, o:o + 1],
                                         scale=pair_w[j][1])
                    nc.sync.dma_start(
                        out=yT[o * P:(o + 1) * P, t * TN:(t + 1) * TN],
                        in_=ot)

                if pair < 2:
                    # x still streaming in: k-chunk outermost so every
                    # arriving x chunk unblocks 8 matmuls (all psum banks)
                    for c in range(KC):
                        for j in range(2):
                            sw = pair_w[j][0]
                            for t in range(TT):
                                nc.tensor.matmul(
                                    ps[j * TT + t],
                                    lhsT=sw[:, c, :],
                                    rhs=x_tiles[c][:, t * TN:(t + 1) * TN],
                                    start=(c == 0),
                                    stop=(c == KC - 1),
                                )
                    for j in range(2):
                        for t in range(TT):
                            epilogue(j, t)
                else:
                    # steady state: one psum group at a time so groups finish
                    # staggered -- banks free incrementally and epilogues
                    # overlap the next group's matmuls
                    for j in range(2):
                        sw = pair_w[j][0]
                        for t in range(TT):
                            for c in range(KC):
                                nc.tensor.matmul(
                                    ps[j * TT + t],
                                    lhsT=sw[:, c, :],
                                    rhs=x_tiles[c][:, t * TN:(t + 1) * TN],
                                    start=(c == 0),
                                    stop=(c == KC - 1),
                                )
                            epilogue(j, t)

                # prefetch next pair's weights (after this pair's matmul
                # emission so the DMAs sit behind x chunks in issue order)
                if pair + 1 < OC // 2:
                    prepped[2 * pair + 2] = weight_prep(2 * pair + 2)
                    prepped[2 * pair + 3] = weight_prep(2 * pair + 3)

    nc.compile()
    _cached_nc = nc
    return nc


def _make_in_maps(x, weight, bias):
    wT = np.ascontiguousarray(weight.T)
    w = np.ascontiguousarray(weight)
    b = np.ascontiguousarray(bias)
    in_maps = []
    for core in range(N_CORES):
        xb = np.ascontiguousarray(x[core].T)  # [in, tok]
        in_maps.append({"xT": xb, "wT": wT, "w": w, "b": b})
    return in_maps


def _setup_trace_hooks():
    """Provide the antenv.axon_hooks NTFF hook missing from this image and
    skip the artifact bucket upload so trace=True works locally."""
    import sys
    import types

    try:
        from antenv.axon_hooks import get_axon_ntff_profile_hook  # noqa: F401
    except ImportError:
        mod = types.ModuleType("antenv.axon_hooks")
        _h = [None]
        mod.set_axon_ntff_profile_hook = lambda h: _h.__setitem__(0, h)
        mod.get_axon_ntff_profile_hook = lambda: _h[0]
        sys.modules["antenv.axon_hooks"] = mod
        import antenv

        antenv.axon_hooks = mod
        from trn_agent_boot.trn_boot import _ntff_profile_via_ctypes

        mod.set_axon_ntff_profile_hook(
            _ntff_profile_via_ctypes("/opt/axon/libaxon_pjrt.so"))

    import concourse.bass_utils as bu

    bu.upload_artifacts = lambda tmpdir: f"local://{tmpdir}"


def kernel(x: np.ndarray, weight: np.ndarray, bias: np.ndarray) -> np.ndarray:
    global last_results
    from concourse.bass_utils import run_bass_kernel_spmd

    x = np.asarray(x, dtype=np.float32)
    weight = np.asarray(weight, dtype=np.float32)
    bias = np.asarray(bias, dtype=np.float32)

    nc = _build_program()
    in_maps = _make_in_maps(x, weight, bias)
    trace = bool(int(os.environ.get("KERNEL_TRACE", "0")))
    trace_cores = None
    if trace:
        _setup_trace_hooks()
        tc_env = os.environ.get("KERNEL_TRACE_CORES", "")
        if tc_env:
            trace_cores = [int(c) for c in tc_env.split(",")]
    res = run_bass_kernel_spmd(nc, in_maps, list(range(N_CORES)), trace=trace,
                               trace_cores=trace_cores)
    last_results = res

    out = np.empty((B, T, O), dtype=np.float32)
    for core in range(N_CORES):
        out[core] = res.results[core]["yT"].T
    return out
